# revision 1
# baseline (speedup 1.0000x reference)
"""EMA scan kernel for Trainium2 (Bass/Tile), 8-core SPMD.

Problem: h_t = (1-a)*y_t + a*h_{t-1}, h_{-1}=0, a=0.9, over y [B=4, S=4096, D=2048] f32.

Sharding: B(4) x D-half(2) -> 8 cores, each core handles a [S=4096, Dc=1024] slab.

Per-core algorithm (exact, matmul-based):
  Split S into 32 blocks of TB=128 rows. For block b:
      h_b = L @ y_b + M1 @ z_{b-1}
  where L[t,j]  = (1-a)*a^(t-j) for t>=j else 0          (in-block causal scan)
        M1[t,j] = (1-a)*a^(t+128-j)                      (previous-block window)
  and z_b = y_b + a^128 * z_{b-1} is a block-level EMA of the raw inputs.
  This is exact because the lag-(128m) window matrices satisfy M_m = a^(128(m-1)) * M1,
  so summing M_m @ y_{b-m} over all m telescopes into M1 @ z_{b-1}.

Since y_b = z_b - c*z_{b-1} (c = a^128), the whole update is rewritten as
    h_b = L@z_b + (M1 - c*L)@z_{b-1}
so every matmul acts on z. TRN2's fp32r matmul (1 cyc/row vs 4 for fp32)
internally rounds operands to 11 mantissa bits but is EXACT when operands
are already on that grid, so each weight W and each z are split once:
Wh=round11(W), Wl=round11(W-Wh) (host consts), zh=round11(z) (an
fp32r-dtype tile write rounds on DVE/GpSimd), zl=z-zh. Then
W@z = Wh@zh + Wh@zl + Wl@zh (the dropped Wl@zl term is ~2^-24). All six
matmuls per 512-column chunk run at 1 cyc/row with fp32-level accuracy
(HW-measured maxabs 4.2e-7 vs the fp64 scan, same as an all-fp32 build).

The fp32 z-chain runs on DVE, zh rounding-copies on GpSimd, zl residuals
on DVE, PSUM->SBUF copies on ACT. Input DMA is batched 2 MiB per transfer
(two 1 MiB groups first to start the pipeline early); output DMA 1 MiB
issued from the ACT HWDGE ring so in/out DMA setup overlaps, with the
last two blocks flushed as 0.5 MiB each to shorten the tail. Dummy warmup
matmuls during the first input DMA hold the PE at full clock (HAM).
Cost-model (TimelineSim) predicted per-core exec: ~103 us against a
~94 us HBM roofline for the 32 MiB/core of traffic.
"""

import numpy as np

import concourse.bass as bass
import concourse.tile as tile
from concourse import bacc, mybir
from concourse import bass_utils

ALPHA = 0.9
B, S, D = 4, 4096, 2048
NCORES = 8
DC = D // 2          # per-core D chunk (1024)
TB = 128             # S-block size (partition dim)
NB = S // TB         # 32 blocks
GK = 4               # blocks per DMA group
NG = NB // GK        # 8 groups
NC_CHUNK = 512       # matmul moving-operand chunk (one PSUM bank, fp32)
F32 = mybir.dt.float32
F32R = mybir.dt.float32r  # tf32-class PE fast path (1 cyc/row vs 4 for fp32)


def _round11(a):
    # round fp32 to 11 explicit mantissa bits (the fp32r-representable grid;
    # such values pass through fp32r matmuls bit-exactly)
    u = np.ascontiguousarray(a, dtype=np.float32).view(np.uint32)
    u2 = ((u + np.uint32(1 << 11)) >> 12) << 12
    return u2.astype(np.uint32).view(np.float32)


def _consts():
    a = ALPHA
    t = np.arange(TB)
    diff = t[:, None] - t[None, :]
    L = np.where(diff >= 0, (1.0 - a) * a ** np.maximum(diff, 0), 0.0)
    M1 = (1.0 - a) * a ** (t[:, None] + TB - t[None, :])
    LT = np.ascontiguousarray(L.T).astype(np.float32)
    M1T = np.ascontiguousarray(M1.T).astype(np.float32)
    c = float(a**TB)
    M1H = _round11(M1T)
    M1LO = _round11(M1T.astype(np.float64) - M1H.astype(np.float64))
    return LT, M1T, c, M1H, M1LO


def _consts2():
    # split2 weight set: h_b = L@z_b + (M1 - c*L)@z_{b-1}, all fp32r via
    # error-free 11-bit splits of both weights and z operands.
    LT, M1T, c, _, _ = _consts()
    LH = _round11(LT)
    LL = _round11(LT.astype(np.float64) - LH.astype(np.float64))
    M1P = M1T.astype(np.float64) - c * LT.astype(np.float64)
    M1PH = _round11(M1P.astype(np.float32))
    M1PL = _round11(M1P - M1PH.astype(np.float64))
    return LH, LL, M1PH, M1PL, c


_CACHE = {}


def _build(ybufs=4, obufs=5, zbufs=5, psbufs=5, gk=GK, dve_frac=0, warmup=6, zsplit=1, out_gk=2, out_eng='scalar', m1_mode='split2', zh_eng='gpsimd', l_first=True, head2=4, tail1=True, tail_f32=False, hsplit=True):
    key = (ybufs, obufs, zbufs, psbufs, gk, dve_frac, warmup, zsplit, out_gk, out_eng, m1_mode, zh_eng, l_first, head2, tail1, tail_f32, hsplit)
    if key in _CACHE:
        return _CACHE[key]
    _, _, c, _, _ = _consts()
    m1_f32 = m1_mode == 'fp32'
    split = m1_mode == 'split'
    split2 = m1_mode == 'split2'
    ZDT = F32 if (m1_f32 or split or split2) else F32R
    GKL = gk
    NGL = NB // gk

    nc = bacc.Bacc(
        "TRN2",
        target_bir_lowering=False,
        debug=False,
        enable_asserts=False,
        num_devices=NCORES,
    )
    y_dram = nc.dram_tensor("y", [S, DC], F32, kind="ExternalInput")
    lt_dram = nc.dram_tensor("lt", [TB, TB], F32, kind="ExternalInput")
    m1t_dram = nc.dram_tensor("m1t", [TB, TB], F32, kind="ExternalInput")
    if split:
        m1l_dram = nc.dram_tensor("m1l", [TB, TB], F32, kind="ExternalInput")
    if split2:
        ll_dram = nc.dram_tensor("ll", [TB, TB], F32, kind="ExternalInput")
        m1l_dram = nc.dram_tensor("m1l", [TB, TB], F32, kind="ExternalInput")
        if tail_f32:
            ltf_dram = nc.dram_tensor("ltf", [TB, TB], F32, kind="ExternalInput")
    out_dram = nc.dram_tensor("out", [S, DC], F32, kind="ExternalOutput")

    with tile.TileContext(nc) as tc:
        with (
            tc.tile_pool(name="consts", bufs=1) as cpool,
            tc.tile_pool(name="ypool", bufs=ybufs) as ypool,
            tc.tile_pool(name="zpool", bufs=zbufs) as zpool,
            tc.tile_pool(name="opool", bufs=obufs) as opool,
            tc.tile_pool(name="zhpool", bufs=zbufs) as zhpool,
            tc.tile_pool(name="zlpool", bufs=zbufs) as zlpool,
            tc.tile_pool(name="psum", bufs=psbufs, space=bass.MemorySpace.PSUM) as pspool,
            tc.tile_pool(name="wps", bufs=1, space=bass.MemorySpace.PSUM) as wpool,
        ):
            lt_sb = cpool.tile([TB, TB], F32R if split2 else F32, tag="lt")
            m1t_sb = cpool.tile([TB, TB], F32 if m1_f32 else F32R, tag="m1t")
            if split2:
                nc.gpsimd.dma_start(lt_sb[:], lt_dram[:])
            else:
                nc.sync.dma_start(lt_sb[:], lt_dram[:])
            if m1_f32:
                nc.sync.dma_start(m1t_sb[:], m1t_dram[:])
            else:
                # SWDGE dma casts fp32 -> fp32r (the verifier requires fp32r
                # matmul operands to be produced pre-rounded; m1 consts are
                # pre-rounded host-side so the cast is bit-exact)
                nc.gpsimd.dma_start(m1t_sb[:], m1t_dram[:])
            if split or split2:
                m1l_sb = cpool.tile([TB, TB], F32R, tag="m1l")
                nc.gpsimd.dma_start(m1l_sb[:], m1l_dram[:])
            if split2:
                ll_sb = cpool.tile([TB, TB], F32R, tag="ll")
                nc.gpsimd.dma_start(ll_sb[:], ll_dram[:])
            if split2 and tail_f32:
                ltf_sb = cpool.tile([TB, TB], F32, tag="ltf")
                nc.sync.dma_start(ltf_sb[:], ltf_dram[:])

            # PE warmup: dummy matmuls on the const tile while the first
            # y-group DMA is in flight, so real matmuls start at full clock
            # (HAM needs ~3us of continuous PE activity).
            if warmup:
                wps = wpool.tile([TB, TB], F32)
                for _ in range(warmup):
                    nc.tensor.matmul(
                        wps[:], lt_sb[:], lt_sb[:], start=True, stop=True
                    )

            zprev = None
            ko_acc = 0
            group_sizes = [2] * head2 + [GKL] * ((NB - 2 * head2) // GKL)
            assert sum(group_sizes) == NB
            gstart = 0
            for g, gsz in enumerate(group_sizes):
                rows = slice(gstart * TB, (gstart + gsz) * TB)
                y_t = ypool.tile([TB, gsz, DC], F32, tag="y_t")
                nc.sync.dma_start(
                    y_t[:], y_dram[rows, :].rearrange("(k p) d -> p k d", k=gsz, p=TB)
                )
                ogk = min(out_gk or gsz, gsz)
                o_t = None
                for k in range(gsz):
                    b = gstart + k
                    cur_ogk = 1 if (tail1 and b >= NB - tail1 * 2) else ogk
                    if ko_acc == 0:
                        o_t = opool.tile([TB, cur_ogk, DC], F32, tag="o_t")
                    ko = ko_acc
                    # block-level EMA of inputs: z_b = y_b + a^128 * z_{b-1}
                    # (split into independent column chunks to shorten the
                    # serial chain; emitted first so DVE dispatches it early)
                    zcur = None
                    if 0 < b < (NB if split2 else NB - 1):
                        z_t = zpool.tile([TB, DC], ZDT)
                        zw = DC // zsplit
                        for zi in range(zsplit):
                            cols = slice(zi * zw, (zi + 1) * zw)
                            zp = zprev[0] if (split or split2) else zprev
                            nc.vector.scalar_tensor_tensor(
                                z_t[:, cols],
                                zp[:, cols],
                                c,
                                y_t[:, k, cols],
                                op0=mybir.AluOpType.mult,
                                op1=mybir.AluOpType.add,
                            )
                        zcur = z_t[:]
                    elif b == 0:
                        if split or split2:
                            zcur = y_t[:, 0, :]
                        else:
                            z_t = zpool.tile([TB, DC], ZDT)
                            nc.vector.tensor_copy(z_t[:], y_t[:, 0, :])
                            zcur = z_t[:]
                    if (split or split2) and zcur is not None and (split2 or b < NB - 1) and not (split2 and tail_f32 and b == NB - 1):
                        # error-free split of z for exact fp32r matmuls:
                        # zh = round11(z) (fp32r write rounds), zl = z - zh.
                        # Split into matmul-chunk halves so each chunk's carry
                        # matmuls start as soon as its half is ready; alternate
                        # zh halves across GpSimd/ACT (both otherwise idle-ish).
                        zh_t = zhpool.tile([TB, DC], F32R)
                        zl_t = zlpool.tile([TB, DC], F32R)
                        halves = (0, NC_CHUNK) if hsplit else (0,)
                        hw_ = NC_CHUNK if hsplit else DC
                        for hi, h0 in enumerate(halves):
                            hs = slice(h0, h0 + hw_)
                            if zh_eng == 'gpsimd':
                                zh_engine = nc.gpsimd
                            elif zh_eng == 'act':
                                zh_engine = nc.scalar
                            else:
                                zh_engine = nc.gpsimd if hi == 0 else nc.scalar
                            if zh_engine is nc.scalar:
                                zh_engine.copy(zh_t[:, hs], zcur[:, hs])
                            else:
                                zh_engine.tensor_copy(zh_t[:, hs], zcur[:, hs])
                            nc.vector.tensor_tensor(
                                zl_t[:, hs],
                                zcur[:, hs],
                                zh_t[:, hs],
                                op=mybir.AluOpType.subtract,
                            )
                        zcur = (zcur, zh_t[:], zl_t[:])
                    for n0 in (0, NC_CHUNK):
                        ps = pspool.tile([TB, NC_CHUNK], F32)
                        rhs_y = y_t[:, k, n0 : n0 + NC_CHUNK]
                        cs = slice(n0, n0 + NC_CHUNK)
                        if split2 and tail_f32 and b == NB - 1:
                            zh_p, zl_p = zprev[1], zprev[2]
                            nc.tensor.matmul(ps[:], m1t_sb[:], zh_p[:, cs], start=True, stop=False)
                            nc.tensor.matmul(ps[:], m1t_sb[:], zl_p[:, cs], start=False, stop=False)
                            nc.tensor.matmul(ps[:], m1l_sb[:], zh_p[:, cs], start=False, stop=False)
                            nc.tensor.matmul(ps[:], ltf_sb[:], zcur[:, cs], start=False, stop=True)
                        elif split2:
                            zh_c, zl_c = zcur[1], zcur[2]
                            if b == 0:
                                nc.tensor.matmul(ps[:], lt_sb[:], zh_c[:, cs], start=True, stop=False)
                                nc.tensor.matmul(ps[:], lt_sb[:], zl_c[:, cs], start=False, stop=False)
                                nc.tensor.matmul(ps[:], ll_sb[:], zh_c[:, cs], start=False, stop=True)
                            else:
                                zh_p, zl_p = zprev[1], zprev[2]
                                nc.tensor.matmul(ps[:], m1t_sb[:], zh_p[:, cs], start=True, stop=False)
                                nc.tensor.matmul(ps[:], m1t_sb[:], zl_p[:, cs], start=False, stop=False)
                                nc.tensor.matmul(ps[:], m1l_sb[:], zh_p[:, cs], start=False, stop=False)
                                nc.tensor.matmul(ps[:], lt_sb[:], zh_c[:, cs], start=False, stop=False)
                                nc.tensor.matmul(ps[:], lt_sb[:], zl_c[:, cs], start=False, stop=False)
                                nc.tensor.matmul(ps[:], ll_sb[:], zh_c[:, cs], start=False, stop=True)
                        elif b == 0:
                            nc.tensor.matmul(ps[:], lt_sb[:], rhs_y, start=True, stop=True)
                        elif split:
                            zh_p, zl_p = zprev[1], zprev[2]
                            if l_first:
                                nc.tensor.matmul(ps[:], lt_sb[:], rhs_y, start=True, stop=False)
                                nc.tensor.matmul(ps[:], m1t_sb[:], zh_p[:, cs], start=False, stop=False)
                                nc.tensor.matmul(ps[:], m1t_sb[:], zl_p[:, cs], start=False, stop=False)
                                nc.tensor.matmul(ps[:], m1l_sb[:], zh_p[:, cs], start=False, stop=True)
                            else:
                                nc.tensor.matmul(ps[:], m1t_sb[:], zh_p[:, cs], start=True, stop=False)
                                nc.tensor.matmul(ps[:], m1t_sb[:], zl_p[:, cs], start=False, stop=False)
                                nc.tensor.matmul(ps[:], m1l_sb[:], zh_p[:, cs], start=False, stop=False)
                                nc.tensor.matmul(ps[:], lt_sb[:], rhs_y, start=False, stop=True)
                        else:
                            # carry matmul in fp32r (tf32-class)
                            nc.tensor.matmul(
                                ps[:], m1t_sb[:], zprev[:, cs], start=True, stop=False
                            )
                            nc.tensor.matmul(ps[:], lt_sb[:], rhs_y, start=False, stop=True)
                        dst = o_t[:, ko, n0 : n0 + NC_CHUNK]
                        if dve_frac and (2 * b + (n0 != 0)) % (dve_frac + 1) < dve_frac:
                            nc.vector.tensor_copy(dst, ps[:])
                        else:
                            nc.scalar.copy(dst, ps[:])
                    if zcur is not None:
                        zprev = zcur
                    ko_acc += 1
                    if ko_acc == cur_ogk:
                        r0 = (b - cur_ogk + 1) * TB
                        orows = slice(r0, r0 + cur_ogk * TB)
                        out_engine = nc.scalar if out_eng == 'scalar' else nc.sync
                        out_engine.dma_start(
                            out_dram[orows, :].rearrange(
                                "(k p) d -> p k d", k=cur_ogk, p=TB
                            ),
                            o_t[:],
                        )
                        ko_acc = 0
                gstart += gsz

    nc.compile()
    _CACHE[key] = nc
    return nc


def kernel(y_seq):
    y_seq = np.asarray(y_seq, dtype=np.float32)
    assert y_seq.shape == (B, S, D), y_seq.shape
    LH, LL, M1PH, M1PL, _ = _consts2()
    nc = _build()

    in_maps = []
    for core in range(NCORES):
        b, h = divmod(core, 2)
        shard = np.ascontiguousarray(y_seq[b, :, h * DC : (h + 1) * DC])
        im = {"y": shard, "lt": LH, "ll": LL, "m1t": M1PH, "m1l": M1PL}
        if "ltf" in {
            a.memorylocations[0].name
            for a in nc.m.functions[0].allocations
            if hasattr(a, "memorylocations") and a.memorylocations
        }:
            im["ltf"] = _consts()[0]
        in_maps.append(im)

    res = None
    for attempt in range(3):
        # transient NRT/device hiccups (e.g. first-exec unrecoverable state)
        # have been observed to succeed on retry
        try:
            res = bass_utils.run_bass_kernel_spmd(
                nc, in_maps, core_ids=list(range(NCORES))
            )
            break
        except Exception:
            if attempt == 2:
                raise
            import time as _time

            _time.sleep(2.0)

    out = np.empty((B, S, D), dtype=np.float32)
    for core in range(NCORES):
        b, h = divmod(core, 2)
        out[b, :, h * DC : (h + 1) * DC] = res.results[core]["out"]
    return out



# revision 3
# speedup vs baseline: 1.9332x; 1.9332x over previous
"""EMA scan kernel for Trainium2 (Bass/Tile), 8-core SPMD.

Problem: h_t = (1-a)*y_t + a*h_{t-1}, h_{-1}=0, a=0.9, over y [B=4, S=4096, D=2048] f32.

Sharding: B(4) x D-half(2) -> 8 cores, each core handles a [S=4096, Dc=1024] slab.

The kernel is HBM-bound (32 MiB/core of f32 I/O against ~360 GB/s), so all
device I/O runs in fp16: the host casts y to fp16 (err ~5e-4), the device
computes h in fp16-in/fp32-accumulate matmuls and writes fp16 h back, and the
host upcasts. That halves DMA traffic to 16 MiB/core (~47 us model floor) and
is far inside the 2e-2 rel-err budget (measured ~2e-4 global L2).

Per-core algorithm: split S into 32 blocks of TB=128 rows. Because alpha^128
= 1.39e-6, contributions older than the previous block are < 2e-6 relative
and are dropped, so each block needs only
    h_b = L @ y_b + M1 @ y_{b-1}
  where L[t,j]  = (1-a)*a^(t-j) for t>=j else 0   (in-block causal scan)
        M1[t,j] = (1-a)*a^(t+128-j)               (previous-block window)
Both matmuls run on the PE in fp16 (1 cyc/row) accumulating in fp32 PSUM;
ACT and DVE each copy one 512-col PSUM chunk to the fp16 staging tile.

All 8 MiB of fp16 input and 8 MiB of fp16 output stay SBUF-resident
(16.1 MiB < 24 MiB), so no tile-pool recycling ever stalls the pipeline:
the 8 input DMAs are issued up-front and per-block output DMAs drain behind
them, keeping the DMA engines (the bottleneck device) gap-free.
"""

import numpy as np

import concourse.bass as bass
import concourse.tile as tile
from concourse import bacc, mybir
from concourse import bass_utils

ALPHA = 0.9
B, S, D = 4, 4096, 2048
NCORES = 8
DC = D // 2          # per-core D chunk (1024)
TB = 128             # S-block size (partition dim)
NB = S // TB         # 32 blocks
NC_CHUNK = 512       # matmul moving-operand chunk (one PSUM bank, fp32)
F32 = mybir.dt.float32
F16 = mybir.dt.float16


def _consts16():
    a = ALPHA
    t = np.arange(TB)
    diff = t[:, None] - t[None, :]
    L = np.where(diff >= 0, (1.0 - a) * a ** np.maximum(diff, 0), 0.0)
    M1 = (1.0 - a) * a ** (t[:, None] + TB - t[None, :])
    LT = np.ascontiguousarray(L.T).astype(np.float16)
    M1T = np.ascontiguousarray(M1.T).astype(np.float16)
    return LT, M1T


_CACHE = {}


def _build(gk=4, psbufs=6, warmup=0, ogk=1):
    key = (gk, psbufs, warmup, ogk)
    if key in _CACHE:
        return _CACHE[key]

    nc = bacc.Bacc(
        "TRN2",
        target_bir_lowering=False,
        debug=False,
        enable_asserts=False,
        num_devices=NCORES,
    )
    y_dram = nc.dram_tensor("y", [S, DC], F16, kind="ExternalInput")
    lt_dram = nc.dram_tensor("lt", [TB, TB], F16, kind="ExternalInput")
    m1t_dram = nc.dram_tensor("m1t", [TB, TB], F16, kind="ExternalInput")
    out_dram = nc.dram_tensor("out", [S, DC], F16, kind="ExternalOutput")

    ng = NB // gk

    with tile.TileContext(nc) as tc:
        with (
            tc.tile_pool(name="consts", bufs=1) as cpool,
            tc.tile_pool(name="ypool", bufs=ng) as ypool,
            tc.tile_pool(name="opool", bufs=NB // ogk) as opool,
            tc.tile_pool(name="psum", bufs=psbufs, space=bass.MemorySpace.PSUM) as pspool,
            tc.tile_pool(name="wps", bufs=1, space=bass.MemorySpace.PSUM) as wpool,
        ):
            lt_sb = cpool.tile([TB, TB], F16, tag="lt")
            m1t_sb = cpool.tile([TB, TB], F16, tag="m1t")
            # consts go through SWDGE (Pool) so SP's HWDGE ring starts on the
            # bulk input immediately
            nc.gpsimd.dma_start(lt_sb[:], lt_dram[:])
            nc.gpsimd.dma_start(m1t_sb[:], m1t_dram[:])

            # all input DMAs issued up-front; whole input is SBUF-resident
            ytiles = []
            for g in range(ng):
                rows = slice(g * gk * TB, (g + 1) * gk * TB)
                y_t = ypool.tile([TB, gk, DC], F16, tag="y_t")
                nc.sync.dma_start(
                    y_t[:],
                    y_dram[rows, :].rearrange("(k p) d -> p k d", k=gk, p=TB),
                )
                ytiles.append(y_t)

            if warmup:
                wps = wpool.tile([TB, TB], F32)
                for _ in range(warmup):
                    nc.tensor.matmul(
                        wps[:], lt_sb[:], lt_sb[:], start=True, stop=True
                    )

            def yb(b):
                g, k = divmod(b, gk)
                return ytiles[g][:, k, :]

            o_t = None
            ko = 0
            for b in range(NB):
                if ko == 0:
                    o_t = opool.tile([TB, ogk, DC], F16, tag="o_t")
                for ci, n0 in enumerate((0, NC_CHUNK)):
                    cs = slice(n0, n0 + NC_CHUNK)
                    ps = pspool.tile([TB, NC_CHUNK], F32)
                    if b == 0:
                        nc.tensor.matmul(
                            ps[:], lt_sb[:], yb(0)[:, cs], start=True, stop=True
                        )
                    else:
                        nc.tensor.matmul(
                            ps[:], m1t_sb[:], yb(b - 1)[:, cs], start=True, stop=False
                        )
                        nc.tensor.matmul(
                            ps[:], lt_sb[:], yb(b)[:, cs], start=False, stop=True
                        )
                    dst = o_t[:, ko, cs]
                    if ci == 0:
                        nc.scalar.copy(dst, ps[:])
                    else:
                        nc.vector.tensor_copy(dst, ps[:])
                ko += 1
                if ko == ogk:
                    r0 = (b - ogk + 1) * TB
                    orows = slice(r0, r0 + ogk * TB)
                    nc.sync.dma_start(
                        out_dram[orows, :].rearrange("(k p) d -> p k d", k=ogk, p=TB),
                        o_t[:],
                    )
                    ko = 0

    nc.compile()
    _CACHE[key] = nc
    return nc


def kernel(y_seq):
    y_seq = np.asarray(y_seq, dtype=np.float32)
    assert y_seq.shape == (B, S, D), y_seq.shape
    LT, M1T = _consts16()
    nc = _build()

    in_maps = []
    for core in range(NCORES):
        b, h = divmod(core, 2)
        shard = np.ascontiguousarray(
            y_seq[b, :, h * DC : (h + 1) * DC].astype(np.float16)
        )
        in_maps.append({"y": shard, "lt": LT, "m1t": M1T})

    res = None
    for attempt in range(3):
        # transient NRT/device hiccups (e.g. first-exec unrecoverable state)
        # have been observed to succeed on retry
        try:
            res = bass_utils.run_bass_kernel_spmd(
                nc, in_maps, core_ids=list(range(NCORES))
            )
            break
        except Exception:
            if attempt == 2:
                raise
            import time as _time

            _time.sleep(2.0)

    out = np.empty((B, S, D), dtype=np.float32)
    for core in range(NCORES):
        b, h = divmod(core, 2)
        out[b, :, h * DC : (h + 1) * DC] = res.results[core]["out"].astype(
            np.float32
        )
    return out


# revision 5
# speedup vs baseline: 1.9915x; 1.0302x over previous
"""EMA scan kernel for Trainium2 (Bass/Tile), 8-core SPMD.

Problem: h_t = (1-a)*y_t + a*h_{t-1}, h_{-1}=0, a=0.9, over y [B=4, S=4096, D=2048] f32.

Sharding: B(4) x D-half(2) -> 8 cores, each core handles a [S=4096, Dc=1024] slab.

The kernel is HBM-bound (32 MiB/core of f32 I/O against ~360 GB/s), so all
device I/O runs in fp16: the host casts y to fp16 (err ~5e-4), the device
computes h in fp16-in/fp32-accumulate matmuls and writes fp16 h back, and the
host upcasts. That halves DMA traffic to 16 MiB/core (~47 us model floor) and
is far inside the 2e-2 rel-err budget (measured ~2e-4 global L2).

Per-core algorithm: split S into 32 blocks of TB=128 rows. Because alpha^128
= 1.39e-6, contributions older than the previous block are < 2e-6 relative
and are dropped, so each block needs only
    h_b = L @ y_b + M1 @ y_{b-1}
  where L[t,j]  = (1-a)*a^(t-j) for t>=j else 0   (in-block causal scan)
        M1[t,j] = (1-a)*a^(t+128-j)               (previous-block window)
Both matmuls run on the PE in fp16 (1 cyc/row) accumulating in fp32 PSUM;
ACT and DVE each copy one 512-col PSUM chunk to the fp16 staging tile.

All 8 MiB of fp16 input and 8 MiB of fp16 output stay SBUF-resident
(16.1 MiB < 24 MiB), so no tile-pool recycling ever stalls the pipeline:
the 8 input DMAs are issued up-front and per-block output DMAs drain behind
them, keeping the DMA engines (the bottleneck device) gap-free.
"""

import numpy as np

import concourse.bass as bass
import concourse.tile as tile
from concourse import bacc, mybir
from concourse import bass_utils

ALPHA = 0.9
B, S, D = 4, 4096, 2048
NCORES = 8
DC = D // 2          # per-core D chunk (1024)
TB = 128             # S-block size (partition dim)
NB = S // TB         # 32 blocks
NC_CHUNK = 512       # matmul moving-operand chunk (one PSUM bank, fp32)
F32 = mybir.dt.float32
F16 = mybir.dt.float16


def _consts16():
    # single [TB, 2*TB] weight tensor: cols [0:TB]=L^T, [TB:2TB]=M1^T
    # (one 512B-elem DMA instead of two 256B-elem ones)
    a = ALPHA
    t = np.arange(TB)
    diff = t[:, None] - t[None, :]
    L = np.where(diff >= 0, (1.0 - a) * a ** np.maximum(diff, 0), 0.0)
    M1 = (1.0 - a) * a ** (t[:, None] + TB - t[None, :])
    W = np.concatenate([L.T, M1.T], axis=1)
    return np.ascontiguousarray(W).astype(np.float16)


_CACHE = {}


def _build(gk=4, psbufs=6, warmup=0, ogk=1):
    key = (gk, psbufs, warmup, ogk)
    if key in _CACHE:
        return _CACHE[key]

    nc = bacc.Bacc(
        "TRN2",
        target_bir_lowering=False,
        debug=False,
        enable_asserts=False,
        num_devices=NCORES,
    )
    y_dram = nc.dram_tensor("y", [S, DC], F16, kind="ExternalInput")
    w_dram = nc.dram_tensor("w", [TB, 2 * TB], F16, kind="ExternalInput")
    out_dram = nc.dram_tensor("out", [S, DC], F16, kind="ExternalOutput")

    ng = NB // gk

    with tile.TileContext(nc) as tc:
        with (
            tc.tile_pool(name="consts", bufs=1) as cpool,
            tc.tile_pool(name="ypool", bufs=ng) as ypool,
            tc.tile_pool(name="opool", bufs=NB // ogk) as opool,
            tc.tile_pool(name="psum", bufs=psbufs, space=bass.MemorySpace.PSUM) as pspool,
        ):
            w_sb = cpool.tile([TB, 2 * TB], F16, tag="w")
            lt_sb = w_sb[:, :TB]
            m1t_sb = w_sb[:, TB : 2 * TB]

            # all input DMAs issued up-front on SP/HWDGE; whole input is
            # SBUF-resident. Weights go second: their 182ns transfer slots
            # right after in_0 and they are only needed once block 1 starts.
            ytiles = []
            for g in range(ng):
                rows = slice(g * gk * TB, (g + 1) * gk * TB)
                y_t = ypool.tile([TB, gk, DC], F16, tag="y_t")
                nc.sync.dma_start(
                    y_t[:],
                    y_dram[rows, :].rearrange("(k p) d -> p k d", k=gk, p=TB),
                )
                ytiles.append(y_t)
                if g == 0:
                    nc.sync.dma_start(w_sb[:], w_dram[:])

            def yb(b):
                g, k = divmod(b, gk)
                return ytiles[g][:, k, :]

            o_t = None
            ko = 0
            for b in range(NB):
                if ko == 0:
                    o_t = opool.tile([TB, ogk, DC], F16, tag="o_t")
                for ci, n0 in enumerate((0, NC_CHUNK)):
                    cs = slice(n0, n0 + NC_CHUNK)
                    ps_t = pspool.tile([TB, NC_CHUNK], F32, tag="ps")
                    ps = ps_t[:]
                    if b == 0:
                        nc.tensor.matmul(
                            ps, lt_sb, yb(0)[:, cs], start=True, stop=True
                        )
                    else:
                        nc.tensor.matmul(
                            ps, m1t_sb, yb(b - 1)[:, cs], start=True, stop=False
                        )
                        nc.tensor.matmul(
                            ps, lt_sb, yb(b)[:, cs], start=False, stop=True
                        )
                    dst = o_t[:, ko, cs]
                    if ci == 0:
                        nc.scalar.copy(dst, ps)
                    else:
                        nc.vector.tensor_copy(dst, ps)
                ko += 1
                if ko == ogk:
                    r0 = (b - ogk + 1) * TB
                    orows = slice(r0, r0 + ogk * TB)
                    nc.sync.dma_start(
                        out_dram[orows, :].rearrange("(k p) d -> p k d", k=ogk, p=TB),
                        o_t[:],
                    )
                    ko = 0

    nc.compile()
    _CACHE[key] = nc
    return nc


def kernel(y_seq):
    y_seq = np.asarray(y_seq, dtype=np.float32)
    assert y_seq.shape == (B, S, D), y_seq.shape
    W = _consts16()
    nc = _build()

    in_maps = []
    for core in range(NCORES):
        b, h = divmod(core, 2)
        shard = np.ascontiguousarray(
            y_seq[b, :, h * DC : (h + 1) * DC].astype(np.float16)
        )
        in_maps.append({"y": shard, "w": W})

    res = None
    for attempt in range(3):
        # transient NRT/device hiccups (e.g. first-exec unrecoverable state)
        # have been observed to succeed on retry
        try:
            res = bass_utils.run_bass_kernel_spmd(
                nc, in_maps, core_ids=list(range(NCORES))
            )
            break
        except Exception:
            if attempt == 2:
                raise
            import time as _time

            _time.sleep(2.0)

    out = np.empty((B, S, D), dtype=np.float32)
    for core in range(NCORES):
        b, h = divmod(core, 2)
        out[b, :, h * DC : (h + 1) * DC] = res.results[core]["out"].astype(
            np.float32
        )
    return out


# revision 7
# speedup vs baseline: 2.0027x; 1.0056x over previous
"""EMA scan kernel for Trainium2 (Bass/Tile), 8-core SPMD.

Problem: h_t = (1-a)*y_t + a*h_{t-1}, h_{-1}=0, a=0.9, over y [B=4, S=4096, D=2048] f32.

Sharding: B(4) x D-half(2) -> 8 cores, each core handles a [S=4096, Dc=1024] slab.

The kernel is HBM-bound (32 MiB/core of f32 I/O against ~360 GB/s), so all
device I/O runs in fp16: the host casts y to fp16 (err ~5e-4), the device
computes h in fp16-in/fp32-accumulate matmuls and writes fp16 h back, and the
host upcasts. That halves DMA traffic to 16 MiB/core (~47 us model floor) and
is far inside the 2e-2 rel-err budget (measured ~2e-4 global L2).

Per-core algorithm: split S into 32 blocks of TB=128 rows. Because alpha^128
= 1.39e-6, contributions older than the previous block are < 2e-6 relative
and are dropped, so each block needs only
    h_b = L @ y_b + M1 @ y_{b-1}
  where L[t,j]  = (1-a)*a^(t-j) for t>=j else 0   (in-block causal scan)
        M1[t,j] = (1-a)*a^(t+128-j)               (previous-block window)
Both matmuls run on the PE in fp16 (1 cyc/row) accumulating in fp32 PSUM;
ACT and DVE each copy one 512-col PSUM chunk to the fp16 staging tile.

All 8 MiB of fp16 input and 8 MiB of fp16 output stay SBUF-resident
(16.1 MiB < 24 MiB), so no tile-pool recycling ever stalls the pipeline:
the 8 input DMAs are issued up-front and per-block output DMAs drain behind
them, keeping the DMA engines (the bottleneck device) gap-free.
"""

import numpy as np

import concourse.bass as bass
import concourse.tile as tile
from concourse import bacc, mybir
from concourse import bass_utils

ALPHA = 0.9
B, S, D = 4, 4096, 2048
NCORES = 8
DC = D // 2          # per-core D chunk (1024)
TB = 128             # S-block size (partition dim)
NB = S // TB         # 32 blocks
NC_CHUNK = 512       # matmul moving-operand chunk (one PSUM bank, fp32)
F32 = mybir.dt.float32
F16 = mybir.dt.float16


def _consts16():
    # host-side reference copy of the on-device weight tensor, for checking:
    # cols [0:TB] = L^T (unscaled by 1-a), [TB:2TB] = M1^T
    a = ALPHA
    t = np.arange(TB)
    diff = t[:, None] - t[None, :]
    L = np.where(diff >= 0, a ** np.maximum(diff, 0), 0.0)
    M1 = a ** (t[:, None] + TB - t[None, :])
    W = np.concatenate([L.T, M1.T], axis=1)
    return np.ascontiguousarray(W).astype(np.float16)


_CACHE = {}


def _build(gk=4, psbufs=6, warmup=0, ogk=1):
    key = (gk, psbufs, warmup, ogk)
    if key in _CACHE:
        return _CACHE[key]

    nc = bacc.Bacc(
        "TRN2",
        target_bir_lowering=False,
        debug=False,
        enable_asserts=False,
        num_devices=NCORES,
    )
    y_dram = nc.dram_tensor("y", [S, DC], F16, kind="ExternalInput")
    out_dram = nc.dram_tensor("out", [S, DC], F16, kind="ExternalOutput")

    ng = NB // gk

    with tile.TileContext(nc) as tc:
        with (
            tc.tile_pool(name="consts", bufs=1) as cpool,
            tc.tile_pool(name="ypool", bufs=ng) as ypool,
            tc.tile_pool(name="opool", bufs=NB // ogk) as opool,
            tc.tile_pool(name="psum", bufs=psbufs, space=bass.MemorySpace.PSUM) as pspool,
        ):
            # weights are generated ON DEVICE (no DMA): W[j, c] = a^(c-j)
            # for both halves -- cols [0:TB] are L^T (masked to upper-tri),
            # cols [TB:2TB] are M1^T since M1^T[j,t] = a^((t+TB)-j).
            # The (1-a) prefactor is folded into the PSUM->SBUF copies.
            xw = cpool.tile([TB, 2 * TB], F32, tag="xw")
            w_sb = cpool.tile([TB, 2 * TB], F16, tag="w")
            lt_sb = w_sb[:, :TB]
            m1t_sb = w_sb[:, TB : 2 * TB]
            nc.gpsimd.iota(
                xw[:],
                pattern=[[1, 2 * TB]],
                base=0,
                channel_multiplier=-1,
                allow_small_or_imprecise_dtypes=True,
            )
            # causal mask for the L half: exponent < 0 -> +1e4, which after
            # the Exp(x * ln(alpha)) with ln(alpha) < 0 underflows to 0.0
            nc.gpsimd.affine_select(
                xw[:, :TB],
                xw[:, :TB],
                pattern=[[1, TB]],
                compare_op=mybir.AluOpType.is_ge,
                fill=1e4,
                base=0,
                channel_multiplier=-1,
            )
            nc.scalar.activation(
                w_sb[:], xw[:], mybir.ActivationFunctionType.Exp,
                scale=float(np.log(ALPHA)),
            )

            # all input DMAs issued up-front on SP/HWDGE; whole input is
            # SBUF-resident
            ytiles = []
            for g in range(ng):
                rows = slice(g * gk * TB, (g + 1) * gk * TB)
                y_t = ypool.tile([TB, gk, DC], F16, tag="y_t")
                nc.sync.dma_start(
                    y_t[:],
                    y_dram[rows, :].rearrange("(k p) d -> p k d", k=gk, p=TB),
                )
                ytiles.append(y_t)

            def yb(b):
                g, k = divmod(b, gk)
                return ytiles[g][:, k, :]

            o_t = None
            ko = 0
            for b in range(NB):
                if ko == 0:
                    o_t = opool.tile([TB, ogk, DC], F16, tag="o_t")
                for ci, n0 in enumerate((0, NC_CHUNK)):
                    cs = slice(n0, n0 + NC_CHUNK)
                    ps_t = pspool.tile([TB, NC_CHUNK], F32, tag="ps")
                    ps = ps_t[:]
                    if b == 0:
                        nc.tensor.matmul(
                            ps, lt_sb, yb(0)[:, cs], start=True, stop=True
                        )
                    else:
                        nc.tensor.matmul(
                            ps, m1t_sb, yb(b - 1)[:, cs], start=True, stop=False
                        )
                        nc.tensor.matmul(
                            ps, lt_sb, yb(b)[:, cs], start=False, stop=True
                        )
                    # (1-a) scaling folded in here
                    dst = o_t[:, ko, cs]
                    if ci == 0:
                        nc.scalar.activation(
                            dst, ps, mybir.ActivationFunctionType.Copy,
                            scale=1.0 - ALPHA,
                        )
                    else:
                        nc.vector.tensor_scalar(
                            dst, ps, 1.0 - ALPHA, None,
                            op0=mybir.AluOpType.mult,
                        )
                ko += 1
                if ko == ogk:
                    r0 = (b - ogk + 1) * TB
                    orows = slice(r0, r0 + ogk * TB)
                    nc.sync.dma_start(
                        out_dram[orows, :].rearrange("(k p) d -> p k d", k=ogk, p=TB),
                        o_t[:],
                    )
                    ko = 0

    nc.compile()
    _CACHE[key] = nc
    return nc


def kernel(y_seq):
    y_seq = np.asarray(y_seq, dtype=np.float32)
    assert y_seq.shape == (B, S, D), y_seq.shape
    nc = _build()

    in_maps = []
    for core in range(NCORES):
        b, h = divmod(core, 2)
        shard = np.ascontiguousarray(
            y_seq[b, :, h * DC : (h + 1) * DC].astype(np.float16)
        )
        in_maps.append({"y": shard})

    res = None
    for attempt in range(3):
        # transient NRT/device hiccups (e.g. first-exec unrecoverable state)
        # have been observed to succeed on retry
        try:
            res = bass_utils.run_bass_kernel_spmd(
                nc, in_maps, core_ids=list(range(NCORES))
            )
            break
        except Exception:
            if attempt == 2:
                raise
            import time as _time

            _time.sleep(2.0)

    out = np.empty((B, S, D), dtype=np.float32)
    for core in range(NCORES):
        b, h = divmod(core, 2)
        out[b, :, h * DC : (h + 1) * DC] = res.results[core]["out"].astype(
            np.float32
        )
    return out


# revision 8
# speedup vs baseline: 2.0031x; 1.0002x over previous
"""EMA scan kernel for Trainium2 (Bass/Tile), 8-core SPMD.

Problem: h_t = (1-a)*y_t + a*h_{t-1}, h_{-1}=0, a=0.9, over y [B=4, S=4096, D=2048] f32.

Sharding: B(4) x D-half(2) -> 8 cores, each core handles a [S=4096, Dc=1024] slab.

The kernel is HBM-bound (32 MiB/core of f32 I/O against ~360 GB/s), so all
device I/O runs in fp16: the host casts y to fp16 (err ~5e-4), the device
computes h in fp16-in/fp32-accumulate matmuls and writes fp16 h back, and the
host upcasts. That halves DMA traffic to 16 MiB/core (~47 us model floor) and
is far inside the 2e-2 rel-err budget (measured ~2e-4 global L2).

Per-core algorithm: split S into 32 blocks of TB=128 rows. Because alpha^128
= 1.39e-6, contributions older than the previous block are < 2e-6 relative
and are dropped, so each block needs only
    h_b = L @ y_b + M1 @ y_{b-1}
  where L[t,j]  = (1-a)*a^(t-j) for t>=j else 0   (in-block causal scan)
        M1[t,j] = (1-a)*a^(t+128-j)               (previous-block window)
Both matmuls run on the PE in fp16 (1 cyc/row) accumulating in fp32 PSUM;
ACT and DVE each copy one 512-col PSUM chunk to the fp16 staging tile.

All 8 MiB of fp16 input and 8 MiB of fp16 output stay SBUF-resident
(16.1 MiB < 24 MiB), so no tile-pool recycling ever stalls the pipeline:
the 8 input DMAs are issued up-front and per-block output DMAs drain behind
them, keeping the DMA engines (the bottleneck device) gap-free.
"""

import numpy as np

import concourse.bass as bass
import concourse.tile as tile
from concourse import bacc, mybir
from concourse import bass_utils

ALPHA = 0.9
B, S, D = 4, 4096, 2048
NCORES = 8
DC = D // 2          # per-core D chunk (1024)
TB = 128             # S-block size (partition dim)
NB = S // TB         # 32 blocks
NC_CHUNK = 512       # matmul moving-operand chunk (one PSUM bank, fp32)
F32 = mybir.dt.float32
F16 = mybir.dt.float16


def _consts16():
    # host-side reference copy of the on-device weight tensor, for checking:
    # cols [0:TB] = L^T (unscaled by 1-a), [TB:2TB] = M1^T
    a = ALPHA
    t = np.arange(TB)
    diff = t[:, None] - t[None, :]
    L = np.where(diff >= 0, a ** np.maximum(diff, 0), 0.0)
    M1 = a ** (t[:, None] + TB - t[None, :])
    W = np.concatenate([L.T, M1.T], axis=1)
    return np.ascontiguousarray(W).astype(np.float16)


_CACHE = {}


def _build(gk=2, psbufs=6, warmup=0, ogk=1):
    key = (gk, psbufs, warmup, ogk)
    if key in _CACHE:
        return _CACHE[key]

    nc = bacc.Bacc(
        "TRN2",
        target_bir_lowering=False,
        debug=False,
        enable_asserts=False,
        num_devices=NCORES,
    )
    y_dram = nc.dram_tensor("y", [S, DC], F16, kind="ExternalInput")
    out_dram = nc.dram_tensor("out", [S, DC], F16, kind="ExternalOutput")

    ng = NB // gk

    with tile.TileContext(nc) as tc:
        with (
            tc.tile_pool(name="consts", bufs=1) as cpool,
            tc.tile_pool(name="ypool", bufs=ng) as ypool,
            tc.tile_pool(name="opool", bufs=NB // ogk) as opool,
            tc.tile_pool(name="psum", bufs=psbufs, space=bass.MemorySpace.PSUM) as pspool,
        ):
            # weights are generated ON DEVICE (no DMA): W[j, c] = a^(c-j)
            # for both halves -- cols [0:TB] are L^T (masked to upper-tri),
            # cols [TB:2TB] are M1^T since M1^T[j,t] = a^((t+TB)-j).
            # The (1-a) prefactor is folded into the PSUM->SBUF copies.
            xw = cpool.tile([TB, 2 * TB], F32, tag="xw")
            w_sb = cpool.tile([TB, 2 * TB], F16, tag="w")
            lt_sb = w_sb[:, :TB]
            m1t_sb = w_sb[:, TB : 2 * TB]
            nc.gpsimd.iota(
                xw[:],
                pattern=[[1, 2 * TB]],
                base=0,
                channel_multiplier=-1,
                allow_small_or_imprecise_dtypes=True,
            )
            # causal mask for the L half: exponent < 0 -> +1e4, which after
            # the Exp(x * ln(alpha)) with ln(alpha) < 0 underflows to 0.0
            nc.gpsimd.affine_select(
                xw[:, :TB],
                xw[:, :TB],
                pattern=[[1, TB]],
                compare_op=mybir.AluOpType.is_ge,
                fill=1e4,
                base=0,
                channel_multiplier=-1,
            )
            nc.scalar.activation(
                w_sb[:], xw[:], mybir.ActivationFunctionType.Exp,
                scale=float(np.log(ALPHA)),
            )

            # all input DMAs issued up-front on SP/HWDGE; whole input is
            # SBUF-resident
            ytiles = []
            for g in range(ng):
                rows = slice(g * gk * TB, (g + 1) * gk * TB)
                y_t = ypool.tile([TB, gk, DC], F16, tag="y_t")
                nc.sync.dma_start(
                    y_t[:],
                    y_dram[rows, :].rearrange("(k p) d -> p k d", k=gk, p=TB),
                )
                ytiles.append(y_t)

            def yb(b):
                g, k = divmod(b, gk)
                return ytiles[g][:, k, :]

            o_t = None
            ko = 0
            for b in range(NB):
                if ko == 0:
                    o_t = opool.tile([TB, ogk, DC], F16, tag="o_t")
                for ci, n0 in enumerate((0, NC_CHUNK)):
                    cs = slice(n0, n0 + NC_CHUNK)
                    ps_t = pspool.tile([TB, NC_CHUNK], F32, tag="ps")
                    ps = ps_t[:]
                    if b == 0:
                        nc.tensor.matmul(
                            ps, lt_sb, yb(0)[:, cs], start=True, stop=True
                        )
                    else:
                        nc.tensor.matmul(
                            ps, m1t_sb, yb(b - 1)[:, cs], start=True, stop=False
                        )
                        nc.tensor.matmul(
                            ps, lt_sb, yb(b)[:, cs], start=False, stop=True
                        )
                    # (1-a) scaling folded in here
                    dst = o_t[:, ko, cs]
                    if ci == 0:
                        nc.scalar.activation(
                            dst, ps, mybir.ActivationFunctionType.Copy,
                            scale=1.0 - ALPHA,
                        )
                    else:
                        nc.vector.tensor_scalar(
                            dst, ps, 1.0 - ALPHA, None,
                            op0=mybir.AluOpType.mult,
                        )
                ko += 1
                if ko == ogk:
                    r0 = (b - ogk + 1) * TB
                    orows = slice(r0, r0 + ogk * TB)
                    nc.sync.dma_start(
                        out_dram[orows, :].rearrange("(k p) d -> p k d", k=ogk, p=TB),
                        o_t[:],
                    )
                    ko = 0

    nc.compile()
    _CACHE[key] = nc
    return nc


def kernel(y_seq):
    y_seq = np.asarray(y_seq, dtype=np.float32)
    assert y_seq.shape == (B, S, D), y_seq.shape
    nc = _build()

    in_maps = []
    for core in range(NCORES):
        b, h = divmod(core, 2)
        shard = np.ascontiguousarray(
            y_seq[b, :, h * DC : (h + 1) * DC].astype(np.float16)
        )
        in_maps.append({"y": shard})

    res = None
    for attempt in range(3):
        # transient NRT/device hiccups (e.g. first-exec unrecoverable state)
        # have been observed to succeed on retry
        try:
            res = bass_utils.run_bass_kernel_spmd(
                nc, in_maps, core_ids=list(range(NCORES))
            )
            break
        except Exception:
            if attempt == 2:
                raise
            import time as _time

            _time.sleep(2.0)

    out = np.empty((B, S, D), dtype=np.float32)
    for core in range(NCORES):
        b, h = divmod(core, 2)
        out[b, :, h * DC : (h + 1) * DC] = res.results[core]["out"].astype(
            np.float32
        )
    return out
